# revision 1
# baseline (speedup 1.0000x reference)
"""Trainium2 Bass kernel for nn_JunmaiLayer (gnn_message_passing).

Math: h[z,a,o] = sum_{b,d,e,k,c} basis[z,a,b,k,c] * basis[z,d,e,k,c] * W[a,b,d,e,k,o]
      out = silu(h) @ w_fc + b_fc

Factoring used here:
  G[z,k,ab,de] = sum_c basis[z,ab,k,c] * basis[z,de,k,c]      (tiny, host-computed)
  h[z,a,o]    = sum_{b,k,de} G[z,k,ab,de] * W[ab,de,k,o]      (device, streams all of W)

W is 256 MB and each element is used once -> the kernel is HBM-DMA-bound.
Sharding: W split along its leading atom axis `a` across 8 cores (2 atoms each,
32 MB fp32 -> 16 MB fp16 per core). x/basis/G are replicated (G sliced per core).
Each core computes h[z, a_slice, o]; host concatenates (the "all-gather") and
applies the trivial silu+fc epilogue.

Device kernel per core:
  - DMA G slice (fp16, 1 MB) into SBUF once.
  - Stream W slice in 2 MB chunks (4 ab-pairs worth), double-buffered.
  - For each (ab, k, de-half): one matmul  psum[z=4, o=64] +=
        G[de128, z4].T @ W[de128, o64]   accumulated over all 512 matmuls per atom.
  - Copy the two psum tiles [4,64] to SBUF, DMA out h [2,4,64] fp32.

fp16 keeps the tensor engine at 1 cycle/row (fp32 matmul is 4x slower) and
halves DMA bytes; accumulation stays fp32 in PSUM. End-to-end rel err ~2e-4.
"""

import numpy as np

import concourse.bass as bass
import concourse.tile as tile
from concourse import mybir
from concourse.bass_utils import run_bass_kernel_spmd

# ---------------------------------------------------------------- constants
B, N, K, H, O = 4, 16, 16, 64, 1
EPSILON = 1e-5
CUT_LO, CUT_HI = 0.0, 5.0
N_CORES = 8
A_PER_CORE = N // N_CORES          # 2 atoms per core
AB_PER_CORE = A_PER_CORE * N       # 32 (a,b) pairs per core
DE = N * N                         # 256 contraction values, 2 chunks of 128
N_CHUNKS = 8                       # W stream chunks per core (4 ab each, 2 MB fp16)
AB_PER_CHUNK = AB_PER_CORE // N_CHUNKS  # 4

_nc_cache = {}


def _basis_host(x):
    """Replicates reference featurization in float64; returns (B, N*N, K, 3)."""
    x = x.astype(np.float64)
    diff = x[:, :, None, :] - x[:, None, :, :]                # (B,N,N,3)
    norm_sq = np.sum(diff * diff, axis=-1, keepdims=True) + EPSILON
    norm = np.sqrt(norm_sq)
    diffn = diff / norm_sq
    start = np.exp(-CUT_HI + CUT_LO)
    means = np.linspace(start, 1.0, K)
    betas = (2.0 / K * (1.0 - start)) ** -2
    alpha = 5.0 / (CUT_HI - CUT_LO)
    cutoff = 0.5 * (np.cos(np.pi * norm / CUT_HI) + 1.0) * (norm < CUT_HI)
    smear = cutoff * np.exp(-betas * (np.exp(alpha * (-norm + CUT_LO)) - means) ** 2)
    basis = smear[..., None] * diffn[..., None, :]            # (B,N,N,K,3)
    return basis.reshape(B, N * N, K, 3)


def _build_nc():
    """One SPMD Bass program (raw Block API); every core runs its W/G slice.

    Pipeline: sync engine queues G + 8 W-chunk DMAs back-to-back (HWDGE FIFO);
    PE waits per-chunk (own semaphore each -- a shared counting sem can race
    across the 16 SDMA engines) and runs 128 accumulating matmuls per chunk;
    DVE copies the two PSUM results to SBUF; GPSIMD DMAs them out.
    """
    nc = bass.Bass(target_bir_lowering=False)
    # Host pre-arranges W in exact SBUF layout: [p, chunk, q=ab_in_chunk*2+t, ko]
    # so every partition's read per chunk is one contiguous 16 KB block.
    w = nc.dram_tensor("w", [128, N_CHUNKS, AB_PER_CHUNK * 2 * K * H],
                       mybir.dt.float16, kind="ExternalInput")
    g = nc.dram_tensor("g", [128, AB_PER_CORE * K * 2 * B], mybir.dt.float16,
                       kind="ExternalInput")
    h = nc.dram_tensor("h", [B, A_PER_CORE * H], mybir.dt.float32,
                       kind="ExternalOutput")

    import contextlib
    with contextlib.ExitStack() as st:
        gt = st.enter_context(nc.sbuf_tensor(
            "gt", [128, AB_PER_CORE * K * 2 * B], mybir.dt.float16))
        wt = st.enter_context(nc.sbuf_tensor(
            "wt", [128, N_CHUNKS * AB_PER_CHUNK * 2, K * H], mybir.dt.float16))
        ot = st.enter_context(nc.sbuf_tensor(
            "ot", [B, A_PER_CORE * H], mybir.dt.float32))
        ps = [st.enter_context(nc.psum_tensor(f"ps{ai}", [B, H], mybir.dt.float32))
              for ai in range(A_PER_CORE)]
        g_sem = st.enter_context(nc.semaphore("g_sem"))
        w_sems = [st.enter_context(nc.semaphore(f"w_sem{cc}"))
                  for cc in range(N_CHUNKS)]
        w_sem_last = st.enter_context(nc.semaphore("w_sem_last"))
        pe_sem = st.enter_context(nc.semaphore("pe_sem"))
        cp_sem = st.enter_context(nc.semaphore("cp_sem"))
        out_sem = st.enter_context(nc.semaphore("out_sem"))
        block = st.enter_context(nc.Block())

        @block.sync
        def _(sync):
            half = AB_PER_CHUNK * K * H  # elements of half a chunk's free dim
            for cc in range(N_CHUNKS):
                q0 = cc * AB_PER_CHUNK * 2
                if cc < N_CHUNKS - 1:
                    sync.dma_start(
                        wt[:, q0:q0 + AB_PER_CHUNK * 2, :].rearrange(
                            "p q f -> p (q f)"),
                        w[:, cc, :],
                    ).then_inc(w_sems[cc], 16)
                else:
                    # Last chunk split in two so PE overlaps its matmuls
                    # with the second half's DMA (tail latency cut).
                    sync.dma_start(
                        wt[:, q0:q0 + AB_PER_CHUNK, :].rearrange(
                            "p q f -> p (q f)"),
                        w[:, cc, 0:half],
                    ).then_inc(w_sems[cc], 16)
                    sync.dma_start(
                        wt[:, q0 + AB_PER_CHUNK:q0 + AB_PER_CHUNK * 2, :]
                        .rearrange("p q f -> p (q f)"),
                        w[:, cc, half:],
                    ).then_inc(w_sem_last, 16)
            # Output store rides the same HWDGE ring; by the time cp_sem
            # fires the W stream has long drained, so no queuing delay.
            sync.wait_ge(cp_sem, 1)
            sync.dma_start(h[:, :], ot[:, :]).then_inc(out_sem, 16)
            sync.wait_ge(out_sem, 16)

        @block.tensor
        def _(tensor):
            tensor.wait_ge(g_sem, 16)
            for cc in range(N_CHUNKS):
                tensor.wait_ge(w_sems[cc], 16)
                for abin in range(AB_PER_CHUNK):
                    if cc == N_CHUNKS - 1 and abin == AB_PER_CHUNK // 2:
                        tensor.wait_ge(w_sem_last, 16)
                    ab = cc * AB_PER_CHUNK + abin
                    ai = ab // N
                    for k in range(K):
                        for t in range(2):
                            col = ((ab * K + k) * 2 + t) * B
                            stop = (ab % N == N - 1 and k == K - 1 and t == 1)
                            mm = tensor.matmul(
                                ps[ai][:, :],
                                gt[:, col:col + B],
                                wt[:, cc * AB_PER_CHUNK * 2 + abin * 2 + t,
                                   k * H:(k + 1) * H],
                                start=(ab % N == 0 and k == 0 and t == 0),
                                stop=stop,
                            )
                            if stop:
                                mm.then_inc(pe_sem, 1)

        @block.vector
        def _(vector):
            for ai in range(A_PER_CORE):
                vector.wait_ge(pe_sem, ai + 1)
                cp = vector.tensor_copy(
                    out=ot[:, ai * H:(ai + 1) * H], in_=ps[ai][:, :])
                if ai == A_PER_CORE - 1:
                    cp.then_inc(cp_sem, 1)

        @block.gpsimd
        def _(gpsimd):
            # G load on the SWDGE path overlaps W chunk 0 on the HWDGE ring.
            gpsimd.dma_start(gt[:, :], g[:, :]).then_inc(g_sem, 16)
    return nc


def _get_nc():
    if "nc" not in _nc_cache:
        _nc_cache["nc"] = _build_nc()
    return _nc_cache["nc"]


def _make_inputs(x, W):
    bf = _basis_host(x)                                        # (B, 256, K, 3)
    G = np.einsum("zikc,zjkc->zkij", bf, bf)                   # (B, K, 256, 256)
    W16 = np.ascontiguousarray(W, dtype=np.float16).reshape(N, N, DE, K * H)
    in_maps = []
    for c in range(N_CORES):
        wc = W16[c * A_PER_CORE:(c + 1) * A_PER_CORE].reshape(
            AB_PER_CORE, 2, 128, K * H)          # (ab, t, p, f)
        wc = wc.transpose(2, 0, 1, 3).reshape(128, N_CHUNKS, AB_PER_CHUNK * 2 * K * H)
        gc = G[:, :, c * AB_PER_CORE:(c + 1) * AB_PER_CORE, :]  # (B,K,32,256)
        gc = gc.reshape(B, K, AB_PER_CORE, 2, 128)
        gc = gc.transpose(4, 2, 1, 3, 0).reshape(128, AB_PER_CORE * K * 2 * B)
        in_maps.append({
            "w": np.ascontiguousarray(wc),
            "g": np.ascontiguousarray(gc, dtype=np.float16),
        })
    return in_maps


def kernel(x, W, w_fc, b_fc):
    nc = _get_nc()
    in_maps = _make_inputs(x, W)
    res = run_bass_kernel_spmd(nc, in_maps, list(range(N_CORES))).results
    h = np.zeros((B, N, H), dtype=np.float64)
    for c in range(N_CORES):
        hc = res[c]["h"].reshape(B, A_PER_CORE, H)             # (B, 2, H)
        for ai in range(A_PER_CORE):
            h[:, c * A_PER_CORE + ai, :] = hc[:, ai, :]
    sil = h / (1.0 + np.exp(-h))
    out = sil @ w_fc.astype(np.float64) + b_fc.astype(np.float64)
    return out.astype(np.float32)



# revision 3
# speedup vs baseline: 69241.0724x; 69241.0724x over previous
"""Trainium2 Bass kernel for nn_JunmaiLayer (gnn_message_passing).

Math: h[z,a,o] = sum_{b,d,e,k,c} basis[z,a,b,k,c] * basis[z,d,e,k,c] * W[a,b,d,e,k,o]
      out = silu(h) @ w_fc + b_fc

Factoring used here:
  G[z,k,ab,de] = sum_c basis[z,ab,k,c] * basis[z,de,k,c]      (tiny, host-computed)
  h[z,a,o]    = sum_{b,k,de} G[z,k,ab,de] * W[ab,de,k,o]      (device, streams all of W)

W is 256 MB and each element is used once -> the kernel is HBM-DMA-bound.
Sharding: W split along its leading atom axis `a` across 8 cores (2 atoms each,
32 MB fp32 -> 16 MB fp16 per core). x/basis/G are replicated (G sliced per core).
Each core computes h[z, a_slice, o]; host concatenates (the "all-gather") and
applies the trivial silu+fc epilogue.

Device kernel per core:
  - DMA G slice (fp16, 1 MB) into SBUF once.
  - Stream W slice in 2 MB chunks (4 ab-pairs worth), double-buffered.
  - For each (ab, k, de-half): one matmul  psum[z=4, o=64] +=
        G[de128, z4].T @ W[de128, o64]   accumulated over all 512 matmuls per atom.
  - Copy the two psum tiles [4,64] to SBUF, DMA out h [2,4,64] fp32.

fp16 keeps the tensor engine at 1 cycle/row (fp32 matmul is 4x slower) and
halves DMA bytes; accumulation stays fp32 in PSUM. End-to-end rel err ~1e-4.

`_build_nc(repeat=R)` unrolls the identical body R times inside one NEFF
(with cross-iteration WAR guards) so steady-state per-iteration HW time can
be measured from the wall-clock difference of two R values -- NTFF profiling
is unavailable under this axon client.
"""

import numpy as np

import concourse.bass as bass
from concourse import mybir
from concourse.bass_utils import run_bass_kernel_spmd

# ---------------------------------------------------------------- constants
B, N, K, H, O = 4, 16, 16, 64, 1
EPSILON = 1e-5
CUT_LO, CUT_HI = 0.0, 5.0
N_CORES = 8
A_PER_CORE = N // N_CORES          # 2 atoms per core
AB_PER_CORE = A_PER_CORE * N       # 32 (a,b) pairs per core
DE = N * N                         # 256 contraction values, 2 chunks of 128
N_CHUNKS = 8                       # W stream chunks per core (4 ab each, 2 MB fp16)
AB_PER_CHUNK = AB_PER_CORE // N_CHUNKS  # 4

_nc_cache = {}


def _basis_host(x):
    """Replicates reference featurization in float64; returns (B, N*N, K, 3)."""
    x = x.astype(np.float64)
    diff = x[:, :, None, :] - x[:, None, :, :]                # (B,N,N,3)
    norm_sq = np.sum(diff * diff, axis=-1, keepdims=True) + EPSILON
    norm = np.sqrt(norm_sq)
    diffn = diff / norm_sq
    start = np.exp(-CUT_HI + CUT_LO)
    means = np.linspace(start, 1.0, K)
    betas = (2.0 / K * (1.0 - start)) ** -2
    alpha = 5.0 / (CUT_HI - CUT_LO)
    cutoff = 0.5 * (np.cos(np.pi * norm / CUT_HI) + 1.0) * (norm < CUT_HI)
    smear = cutoff * np.exp(-betas * (np.exp(alpha * (-norm + CUT_LO)) - means) ** 2)
    basis = smear[..., None] * diffn[..., None, :]            # (B,N,N,K,3)
    return basis.reshape(B, N * N, K, 3)


def _build_nc(repeat=1):
    """One SPMD Bass program (raw Block API); every core runs its W/G slice.

    Pipeline per iteration: sync engine queues G + 8 W-chunk DMAs
    back-to-back (HWDGE FIFO); PE waits per-chunk and runs 128 accumulating
    matmuls per chunk; DVE copies the two PSUM results to SBUF; sync DMAs
    them out.  With repeat>1 the body is unrolled; cross-iteration waits
    guard every buffer reuse (W chunks, G, PSUM, output staging) so the
    pipeline stays correct while remaining DMA-bound in steady state.
    """
    nc = bass.Bass(target_bir_lowering=False)
    # Host pre-arranges W in exact SBUF layout: [p, chunk, q=ab_in_chunk*2+t, ko]
    # so every partition's read per chunk is one contiguous 16 KB block.
    w = nc.dram_tensor("w", [128, N_CHUNKS, AB_PER_CHUNK * 2 * K * H],
                       mybir.dt.float16, kind="ExternalInput")
    g = nc.dram_tensor("g", [128, AB_PER_CORE * K * 2 * B], mybir.dt.float16,
                       kind="ExternalInput")
    h = nc.dram_tensor("h", [B, A_PER_CORE * H], mybir.dt.float32,
                       kind="ExternalOutput")

    import contextlib
    with contextlib.ExitStack() as st:
        gt = st.enter_context(nc.sbuf_tensor(
            "gt", [128, AB_PER_CORE * K * 2 * B], mybir.dt.float16))
        wt = st.enter_context(nc.sbuf_tensor(
            "wt", [128, N_CHUNKS * AB_PER_CHUNK * 2, K * H], mybir.dt.float16))
        ot = st.enter_context(nc.sbuf_tensor(
            "ot", [B, A_PER_CORE * H], mybir.dt.float32))
        ps = [st.enter_context(nc.psum_tensor(f"ps{ai}", [B, H], mybir.dt.float32))
              for ai in range(A_PER_CORE)]
        g_sem = st.enter_context(nc.semaphore("g_sem"))
        w_sems = [st.enter_context(nc.semaphore(f"w_sem{cc}"))
                  for cc in range(N_CHUNKS)]
        w_sem_last = st.enter_context(nc.semaphore("w_sem_last"))
        pe_sem = st.enter_context(nc.semaphore("pe_sem"))      # +1 per chunk consumed
        cp_sem = st.enter_context(nc.semaphore("cp_sem"))      # +1 per atom copied
        out_sem = st.enter_context(nc.semaphore("out_sem"))    # +16 per h store
        block = st.enter_context(nc.Block())

        @block.sync
        def _(sync):
            half = AB_PER_CHUNK * K * H  # elements of half a chunk's free dim
            for it in range(repeat):
                for cc in range(N_CHUNKS):
                    q0 = cc * AB_PER_CHUNK * 2
                    if it > 0:
                        # WAR: chunk cc's SBUF buffer was last consumed by
                        # PE in the previous iteration.
                        sync.wait_ge(pe_sem, N_CHUNKS * (it - 1) + cc + 1)
                    if cc < N_CHUNKS - 1:
                        sync.dma_start(
                            wt[:, q0:q0 + AB_PER_CHUNK * 2, :].rearrange(
                                "p q f -> p (q f)"),
                            w[:, cc, :],
                        ).then_inc(w_sems[cc], 16)
                    else:
                        # Last chunk split in two so PE overlaps its matmuls
                        # with the second half's DMA (tail latency cut).
                        sync.dma_start(
                            wt[:, q0:q0 + AB_PER_CHUNK, :].rearrange(
                                "p q f -> p (q f)"),
                            w[:, cc, 0:half],
                        ).then_inc(w_sems[cc], 16)
                        sync.dma_start(
                            wt[:, q0 + AB_PER_CHUNK:q0 + AB_PER_CHUNK * 2, :]
                            .rearrange("p q f -> p (q f)"),
                            w[:, cc, half:],
                        ).then_inc(w_sem_last, 16)
                # Output store rides the same HWDGE ring; by the time cp_sem
                # fires the W stream has long drained, so no queuing delay.
                sync.wait_ge(cp_sem, A_PER_CORE * (it + 1))
                sync.dma_start(h[:, :], ot[:, :]).then_inc(out_sem, 16)
            sync.wait_ge(out_sem, 16 * repeat)

        @block.tensor
        def _(tensor):
            for it in range(repeat):
                tensor.wait_ge(g_sem, 16 * (it + 1))
                for cc in range(N_CHUNKS):
                    tensor.wait_ge(w_sems[cc], 16 * (it + 1))
                    for abin in range(AB_PER_CHUNK):
                        if cc == N_CHUNKS - 1 and abin == AB_PER_CHUNK // 2:
                            tensor.wait_ge(w_sem_last, 16 * (it + 1))
                        ab = cc * AB_PER_CHUNK + abin
                        ai = ab // N
                        for k in range(K):
                            for t in range(2):
                                start = (ab % N == 0 and k == 0 and t == 0)
                                if start and it > 0:
                                    # WAR: psum[ai] must be copied out of the
                                    # previous iteration before restart.
                                    tensor.wait_ge(
                                        cp_sem, A_PER_CORE * (it - 1) + ai + 1)
                                col = ((ab * K + k) * 2 + t) * B
                                stop = (ab % N == N - 1 and k == K - 1 and t == 1)
                                mm = tensor.matmul(
                                    ps[ai][:, :],
                                    gt[:, col:col + B],
                                    wt[:, cc * AB_PER_CHUNK * 2 + abin * 2 + t,
                                       k * H:(k + 1) * H],
                                    start=start,
                                    stop=stop,
                                )
                                if k == K - 1 and t == 1 and abin == AB_PER_CHUNK - 1:
                                    mm.then_inc(pe_sem, 1)  # chunk consumed

        @block.vector
        def _(vector):
            for it in range(repeat):
                for ai in range(A_PER_CORE):
                    vector.wait_ge(pe_sem, N_CHUNKS * it + (ai + 1) * (N_CHUNKS // A_PER_CORE))
                    if it > 0:
                        # WAR: ot was read by the previous iteration's store.
                        vector.wait_ge(out_sem, 16 * it)
                    vector.tensor_copy(
                        out=ot[:, ai * H:(ai + 1) * H], in_=ps[ai][:, :]
                    ).then_inc(cp_sem, 1)

        @block.gpsimd
        def _(gpsimd):
            for it in range(repeat):
                if it > 0:
                    # WAR: PE finished reading gt for the previous iteration
                    # only once it consumed the last chunk.
                    gpsimd.wait_ge(pe_sem, N_CHUNKS * it)
                # G load on the SWDGE path overlaps W chunk 0 on the HWDGE ring.
                gpsimd.dma_start(gt[:, :], g[:, :]).then_inc(g_sem, 16)
    return nc


def _get_nc(repeat=1):
    key = f"nc{repeat}"
    if key not in _nc_cache:
        _nc_cache[key] = _build_nc(repeat)
    return _nc_cache[key]


def _make_inputs(x, W):
    bf = _basis_host(x)                                        # (B, 256, K, 3)
    G = np.einsum("zikc,zjkc->zkij", bf, bf)                   # (B, K, 256, 256)
    W16 = np.ascontiguousarray(W, dtype=np.float16).reshape(N, N, DE, K * H)
    in_maps = []
    for c in range(N_CORES):
        wc = W16[c * A_PER_CORE:(c + 1) * A_PER_CORE].reshape(
            AB_PER_CORE, 2, 128, K * H)          # (ab, t, p, f)
        wc = wc.transpose(2, 0, 1, 3).reshape(128, N_CHUNKS, AB_PER_CHUNK * 2 * K * H)
        gc = G[:, :, c * AB_PER_CORE:(c + 1) * AB_PER_CORE, :]  # (B,K,32,256)
        gc = gc.reshape(B, K, AB_PER_CORE, 2, 128)
        gc = gc.transpose(4, 2, 1, 3, 0).reshape(128, AB_PER_CORE * K * 2 * B)
        in_maps.append({
            "w": np.ascontiguousarray(wc),
            "g": np.ascontiguousarray(gc, dtype=np.float16),
        })
    return in_maps


def kernel(x, W, w_fc, b_fc):
    nc = _get_nc()
    in_maps = _make_inputs(x, W)
    res = run_bass_kernel_spmd(nc, in_maps, list(range(N_CORES))).results
    h = np.zeros((B, N, H), dtype=np.float64)
    for c in range(N_CORES):
        hc = res[c]["h"].reshape(B, A_PER_CORE, H)             # (B, 2, H)
        for ai in range(A_PER_CORE):
            h[:, c * A_PER_CORE + ai, :] = hc[:, ai, :]
    sil = h / (1.0 + np.exp(-h))
    out = sil @ w_fc.astype(np.float64) + b_fc.astype(np.float64)
    return out.astype(np.float32)


# revision 16
# speedup vs baseline: 119071.8351x; 1.7197x over previous
"""Trainium2 Bass kernel for nn_JunmaiLayer (gnn_message_passing).

Math: h[z,a,o] = sum_{b,d,e,k,c} basis[z,a,b,k,c] * basis[z,d,e,k,c] * W[a,b,d,e,k,o]
      out = silu(h) @ w_fc + b_fc

Factoring used here:
  G[z,k,ab,de] = sum_c basis[z,ab,k,c] * basis[z,de,k,c]      (tiny, host-computed)
  h[z,a,o]    = sum_{b,k,de} G[z,k,ab,de] * W[ab,de,k,o]      (device, streams all of W)

W is 256 MB and each element is used once -> the kernel is HBM-DMA-bound.
Sharding: W split along its leading atom axis `a` across 8 cores (2 atoms each,
32 MB fp32 -> 16 MB fp16 per core). x/basis/G are replicated (G sliced per core).
Each core computes h[z, a_slice, o]; host concatenates (the "all-gather") and
applies the trivial silu+fc epilogue.

Device kernel per core:
  - DMA G slice (fp16, 1 MB) into SBUF once.
  - Stream W slice in 2 MB chunks (4 ab-pairs worth), double-buffered.
  - For each (ab, k, de-half): one matmul  psum[z=4, o=64] +=
        G[de128, z4].T @ W[de128, o64]   accumulated over all 512 matmuls per atom.
  - Copy the two psum tiles [4,64] to SBUF, DMA out h [2,4,64] fp32.

fp16 keeps the tensor engine at 1 cycle/row (fp32 matmul is 4x slower) and
halves DMA bytes; accumulation stays fp32 in PSUM. End-to-end rel err ~1e-4.

`_build_nc(repeat=R)` unrolls the identical body R times inside one NEFF
(with cross-iteration WAR guards) so steady-state per-iteration HW time can
be measured from the wall-clock difference of two R values -- NTFF profiling
is unavailable under this axon client.
"""

import numpy as np

import concourse.bass as bass
from concourse import mybir
from concourse.bass_utils import run_bass_kernel_spmd

# ---------------------------------------------------------------- constants
B, N, K, H, O = 4, 16, 16, 64, 1
EPSILON = 1e-5
CUT_LO, CUT_HI = 0.0, 5.0
N_CORES = 8
A_PER_CORE = N // N_CORES          # 2 atoms per core
AB_PER_CORE = A_PER_CORE * N       # 32 (a,b) pairs per core
DE = N * N                         # 256 contraction values, 2 chunks of 128
N_CHUNKS = 8                       # W stream chunks per core (4 ab each, 2 MB fp16)
AB_PER_CHUNK = AB_PER_CORE // N_CHUNKS  # 4

_nc_cache = {}


def _basis_host(x):
    """Replicates reference featurization in float64; returns (B, N*N, K, 3)."""
    x = x.astype(np.float64)
    diff = x[:, :, None, :] - x[:, None, :, :]                # (B,N,N,3)
    norm_sq = np.sum(diff * diff, axis=-1, keepdims=True) + EPSILON
    norm = np.sqrt(norm_sq)
    diffn = diff / norm_sq
    start = np.exp(-CUT_HI + CUT_LO)
    means = np.linspace(start, 1.0, K)
    betas = (2.0 / K * (1.0 - start)) ** -2
    alpha = 5.0 / (CUT_HI - CUT_LO)
    cutoff = 0.5 * (np.cos(np.pi * norm / CUT_HI) + 1.0) * (norm < CUT_HI)
    smear = cutoff * np.exp(-betas * (np.exp(alpha * (-norm + CUT_LO)) - means) ** 2)
    basis = smear[..., None] * diffn[..., None, :]            # (B,N,N,K,3)
    return basis.reshape(B, N * N, K, 3)


LADDER_OP_F = 2048          # ACT delay-op free dim (f32): 2048 cyc @ 1.2 GHz
LADDER_OPS_PER_TICK = 4     # tick = 4 delay ops + 1 memset  (~6.9 us nominal)


def _build_nc(repeat=1, ladder_ticks=0):
    """One SPMD Bass program (raw Block API); every core runs its W/G slice.

    Pipeline per iteration: sync engine queues G + 8 W-chunk DMAs
    back-to-back (HWDGE FIFO); PE waits per-chunk and runs 128 accumulating
    matmuls per chunk; DVE copies the two PSUM results to SBUF; sync DMAs
    them out.  With repeat>1 the body is unrolled; cross-iteration waits
    guard every buffer reuse (W chunks, G, PSUM, output staging) so the
    pipeline stays correct while remaining DMA-bound in steady state.

    ladder_ticks>0 adds a passive on-device clock on the otherwise-idle
    ScalarE: a free-running ladder of fixed-duration ops that bumps a tick
    counter in SBUF; the DVE snapshots the counter as each iteration
    completes and the snapshots are DMA'd out as `ts` -- per-iteration
    device-side timestamps immune to host/RPC timing noise.
    """
    nc = bass.Bass(target_bir_lowering=False)
    # Host pre-arranges W in exact SBUF layout: [p, chunk, q=ab_in_chunk*2+t, ko]
    # so every partition's read per chunk is one contiguous 16 KB block.
    w = nc.dram_tensor("w", [128, N_CHUNKS, AB_PER_CHUNK * 2 * K * H],
                       mybir.dt.float16, kind="ExternalInput")
    g = nc.dram_tensor("g", [128, AB_PER_CORE * K * 2 * B], mybir.dt.float16,
                       kind="ExternalInput")
    h = nc.dram_tensor("h", [B, A_PER_CORE * H], mybir.dt.float32,
                       kind="ExternalOutput")
    if ladder_ticks:
        ts = nc.dram_tensor("ts", [1, repeat], mybir.dt.float32,
                            kind="ExternalOutput")

    import contextlib
    with contextlib.ExitStack() as st:
        gt = st.enter_context(nc.sbuf_tensor(
            "gt", [128, AB_PER_CORE * K * 2 * B], mybir.dt.float16))
        wt = st.enter_context(nc.sbuf_tensor(
            "wt", [128, N_CHUNKS * AB_PER_CHUNK * 2, K * H], mybir.dt.float16))
        ot = st.enter_context(nc.sbuf_tensor(
            "ot", [B, A_PER_CORE * H], mybir.dt.float32))
        ps = [st.enter_context(nc.psum_tensor(f"ps{ai}", [B, H], mybir.dt.float32))
              for ai in range(A_PER_CORE)]
        if ladder_ticks:
            dlyA = st.enter_context(nc.sbuf_tensor(
                "dlyA", [128, LADDER_OP_F], mybir.dt.float32))
            dlyB = st.enter_context(nc.sbuf_tensor(
                "dlyB", [128, LADDER_OP_F], mybir.dt.float32))
            lt = st.enter_context(nc.sbuf_tensor("lt", [1, 1], mybir.dt.float32))
            one = st.enter_context(nc.sbuf_tensor("one", [1, 1], mybir.dt.float32))
            ob = st.enter_context(nc.sbuf_tensor(
                "ob", [1, repeat], mybir.dt.float32))
        g_sem = st.enter_context(nc.semaphore("g_sem"))
        w_sems = [st.enter_context(nc.semaphore(f"w_sem{cc}"))
                  for cc in range(N_CHUNKS)]
        w_sem_last = st.enter_context(nc.semaphore("w_sem_last"))
        pe_sem = st.enter_context(nc.semaphore("pe_sem"))      # +1 per chunk consumed
        cp_sem = st.enter_context(nc.semaphore("cp_sem"))      # +1 per atom copied
        out_sem = st.enter_context(nc.semaphore("out_sem"))    # +16 per h store
        if ladder_ticks:
            snap_sem = st.enter_context(nc.semaphore("snap_sem"))
            init_sem = st.enter_context(nc.semaphore("init_sem"))
        block = st.enter_context(nc.Block())

        @block.sync
        def _(sync):
            half = AB_PER_CHUNK * K * H  # elements of half a chunk's free dim
            for it in range(repeat):
                for cc in range(N_CHUNKS):
                    q0 = cc * AB_PER_CHUNK * 2
                    if it > 0:
                        # WAR: chunk cc's SBUF buffer was last consumed by
                        # PE in the previous iteration.
                        sync.wait_ge(pe_sem, N_CHUNKS * (it - 1) + cc + 1)
                    if cc < N_CHUNKS - 1:
                        sync.dma_start(
                            wt[:, q0:q0 + AB_PER_CHUNK * 2, :].rearrange(
                                "p q f -> p (q f)"),
                            w[:, cc, :],
                        ).then_inc(w_sems[cc], 16)
                    else:
                        # Last chunk split in two so PE overlaps its matmuls
                        # with the second half's DMA (tail latency cut).
                        sync.dma_start(
                            wt[:, q0:q0 + AB_PER_CHUNK, :].rearrange(
                                "p q f -> p (q f)"),
                            w[:, cc, 0:half],
                        ).then_inc(w_sems[cc], 16)
                        sync.dma_start(
                            wt[:, q0 + AB_PER_CHUNK:q0 + AB_PER_CHUNK * 2, :]
                            .rearrange("p q f -> p (q f)"),
                            w[:, cc, half:],
                        ).then_inc(w_sem_last, 16)
                # Output store rides the same HWDGE ring; by the time cp_sem
                # fires the W stream has long drained, so no queuing delay.
                sync.wait_ge(cp_sem, A_PER_CORE * (it + 1))
                sync.dma_start(h[:, :], ot[:, :]).then_inc(out_sem, 16)
            if ladder_ticks:
                sync.wait_ge(snap_sem, repeat)
                sync.dma_start(ts[:, :], ob[:, :]).then_inc(out_sem, 16)
                sync.wait_ge(out_sem, 16 * repeat + 16)
            else:
                sync.wait_ge(out_sem, 16 * repeat)

        if ladder_ticks:
            @block.scalar
            def _(scalar):
                scalar.wait_ge(init_sem, 1)
                for m in range(1, ladder_ticks + 1):
                    for _ in range(LADDER_OPS_PER_TICK):
                        scalar.activation(
                            dlyB[:, :], dlyA[:, :],
                            mybir.ActivationFunctionType.Copy)
                    # lt = one * m  (ACT has no memset; Copy-with-scale)
                    scalar.mul(lt[:, :], one[:, :], float(m))

        @block.tensor
        def _(tensor):
            for it in range(repeat):
                tensor.wait_ge(g_sem, 16 * (it + 1))
                for cc in range(N_CHUNKS):
                    tensor.wait_ge(w_sems[cc], 16 * (it + 1))
                    for abin in range(AB_PER_CHUNK):
                        if cc == N_CHUNKS - 1 and abin == AB_PER_CHUNK // 2:
                            tensor.wait_ge(w_sem_last, 16 * (it + 1))
                        ab = cc * AB_PER_CHUNK + abin
                        ai = ab // N
                        for k in range(K):
                            for t in range(2):
                                start = (ab % N == 0 and k == 0 and t == 0)
                                if start and it > 0:
                                    # WAR: psum[ai] must be copied out of the
                                    # previous iteration before restart.
                                    tensor.wait_ge(
                                        cp_sem, A_PER_CORE * (it - 1) + ai + 1)
                                col = ((ab * K + k) * 2 + t) * B
                                stop = (ab % N == N - 1 and k == K - 1 and t == 1)
                                mm = tensor.matmul(
                                    ps[ai][:, :],
                                    gt[:, col:col + B],
                                    wt[:, cc * AB_PER_CHUNK * 2 + abin * 2 + t,
                                       k * H:(k + 1) * H],
                                    start=start,
                                    stop=stop,
                                )
                                if k == K - 1 and t == 1 and abin == AB_PER_CHUNK - 1:
                                    mm.then_inc(pe_sem, 1)  # chunk consumed

        @block.vector
        def _(vector):
            if ladder_ticks:
                vector.memset(lt[:, :], 0.0)
                vector.memset(one[:, :], 1.0).then_inc(init_sem, 1)
            for it in range(repeat):
                for ai in range(A_PER_CORE):
                    vector.wait_ge(pe_sem, N_CHUNKS * it + (ai + 1) * (N_CHUNKS // A_PER_CORE))
                    if it > 0:
                        # WAR: ot was read by the previous iteration's store.
                        vector.wait_ge(out_sem, 16 * it)
                    vector.tensor_copy(
                        out=ot[:, ai * H:(ai + 1) * H], in_=ps[ai][:, :]
                    ).then_inc(cp_sem, 1)
                if ladder_ticks:
                    # snapshot the ladder tick counter: device timestamp of
                    # this iteration's completion
                    vector.tensor_copy(
                        out=ob[:, it:it + 1], in_=lt[:, :]
                    ).then_inc(snap_sem, 1)

        @block.gpsimd
        def _(gpsimd):
            for it in range(repeat):
                if it > 0:
                    # WAR: PE finished reading gt for the previous iteration
                    # only once it consumed the last chunk.
                    gpsimd.wait_ge(pe_sem, N_CHUNKS * it)
                # G load on the SWDGE path overlaps W chunk 0 on the HWDGE ring.
                gpsimd.dma_start(gt[:, :], g[:, :]).then_inc(g_sem, 16)
    return nc


def _get_nc(repeat=1, ladder_ticks=0):
    key = f"nc{repeat}_{ladder_ticks}"
    if key not in _nc_cache:
        _nc_cache[key] = _build_nc(repeat, ladder_ticks)
    return _nc_cache[key]


def _build_ladder_only(ticks):
    """Calibration program: just the ScalarE tick ladder, nothing else.
    Wall-clock slope between two tick counts measures the tick duration."""
    nc = bass.Bass(target_bir_lowering=False)
    a = nc.dram_tensor("a", [1, 16], mybir.dt.float32, kind="ExternalInput")
    ts = nc.dram_tensor("ts", [1, 1], mybir.dt.float32, kind="ExternalOutput")
    import contextlib
    with contextlib.ExitStack() as st:
        dlyA = st.enter_context(nc.sbuf_tensor(
            "dlyA", [128, LADDER_OP_F], mybir.dt.float32))
        dlyB = st.enter_context(nc.sbuf_tensor(
            "dlyB", [128, LADDER_OP_F], mybir.dt.float32))
        lt = st.enter_context(nc.sbuf_tensor("lt", [1, 1], mybir.dt.float32))
        one = st.enter_context(nc.sbuf_tensor("one", [1, 1], mybir.dt.float32))
        sem = st.enter_context(nc.semaphore("sem"))
        init_sem = st.enter_context(nc.semaphore("init_sem"))
        block = st.enter_context(nc.Block())

        @block.vector
        def _(vector):
            vector.memset(lt[:, :], 0.0)
            vector.memset(one[:, :], 1.0).then_inc(init_sem, 1)

        @block.scalar
        def _(scalar):
            scalar.wait_ge(init_sem, 1)
            for m in range(1, ticks + 1):
                for _ in range(LADDER_OPS_PER_TICK):
                    scalar.activation(
                        dlyB[:, :], dlyA[:, :],
                        mybir.ActivationFunctionType.Copy)
                scalar.mul(lt[:, :], one[:, :], float(m))
            scalar.dma_start(ts[:, :], lt[:, :]).then_inc(sem, 16)
            scalar.wait_ge(sem, 16)
    return nc


def _make_inputs(x, W):
    bf = _basis_host(x)                                        # (B, 256, K, 3)
    G = np.einsum("zikc,zjkc->zkij", bf, bf)                   # (B, K, 256, 256)
    W16 = np.ascontiguousarray(W, dtype=np.float16).reshape(N, N, DE, K * H)
    in_maps = []
    for c in range(N_CORES):
        wc = W16[c * A_PER_CORE:(c + 1) * A_PER_CORE].reshape(
            AB_PER_CORE, 2, 128, K * H)          # (ab, t, p, f)
        wc = wc.transpose(2, 0, 1, 3).reshape(128, N_CHUNKS, AB_PER_CHUNK * 2 * K * H)
        gc = G[:, :, c * AB_PER_CORE:(c + 1) * AB_PER_CORE, :]  # (B,K,32,256)
        gc = gc.reshape(B, K, AB_PER_CORE, 2, 128)
        gc = gc.transpose(4, 2, 1, 3, 0).reshape(128, AB_PER_CORE * K * 2 * B)
        in_maps.append({
            "w": np.ascontiguousarray(wc),
            "g": np.ascontiguousarray(gc, dtype=np.float16),
        })
    return in_maps


def kernel(x, W, w_fc, b_fc):
    nc = _get_nc()
    in_maps = _make_inputs(x, W)
    res = run_bass_kernel_spmd(nc, in_maps, list(range(N_CORES))).results
    h = np.zeros((B, N, H), dtype=np.float64)
    for c in range(N_CORES):
        hc = res[c]["h"].reshape(B, A_PER_CORE, H)             # (B, 2, H)
        for ai in range(A_PER_CORE):
            h[:, c * A_PER_CORE + ai, :] = hc[:, ai, :]
    sil = h / (1.0 + np.exp(-h))
    out = sil @ w_fc.astype(np.float64) + b_fc.astype(np.float64)
    return out.astype(np.float32)
